# revision 1
# baseline (speedup 1.0000x reference)
"""JointLoss (YOLO-style bbox + landmarks + confidence) on 8 Trainium2 cores.

Strategy: the three losses only read predictions at obj cells (<= B*T = 1024
of the 207360 grid cells) except the confidence term, which needs
sum(conf^2) over the whole grid.  Host builds the target assignment (tiny:
32x32 IoU argmax + scatter, replicated bit-exactly with jax-CPU), gathers
the obj-cell rows, and ships per-core: the core's bbox-prediction slab (for
the dense conf reduction) + gathered rows packed into one tensor.  Device
(data-parallel over batch, 4 batches/core) computes per-partition partial
sums; host combines in f64.

Raw Bass (no TileContext: its multi-wait tail drain does not compile on
this walrus build).  Explicit semaphores; DVE write-buffer drains between
dependent op levels (raw Bass does not auto-insert them).
"""

import numpy as np

B, T, G, A = 32, 32, 36, 5
NCORES = 8
BPC = B // NCORES            # batches per core
CELLS = G * G * A            # 6480 per batch
ROWS = BPC * T               # max obj rows per core = 128
SLAB_P, SLAB_F = 120, 216    # 120 * 216 * 5 == BPC * CELLS * 5 == 129600
CONF_F = 204                 # ceil(BPC*CELLS/128): conf channel, zero-padded
SMALL_F = 284 + CONF_F       # 136 lmp + 136 lmt + 4 bbp + 4 bbt + 4 aux + conf

IMAGE_SIZE = 288.0
ANCHORS = np.array([[0.24, 0.24], [0.12, 0.12], [0.08, 0.08],
                    [0.28, 0.28], [0.15, 0.15]], dtype=np.float32)

_STATE = {}


def _build_program():
    import concourse.bass as bass
    from concourse import mybir
    from contextlib import ExitStack

    nc = bass.Bass()
    f32 = mybir.dt.float32
    small_p = nc.declare_dram_parameter("small", [ROWS, 284], f32, isOutput=False)
    conf_p = nc.declare_dram_parameter("conf", [ROWS, CONF_F], f32, isOutput=False)
    out_p = nc.declare_dram_parameter("out", [ROWS, 8], f32, isOutput=True)

    st = ExitStack()
    Tt = lambda n, s: st.enter_context(nc.sbuf_tensor(n, s, f32))
    small_t = Tt("small_t", [ROWS, 284])
    conf_t = Tt("conf_t", [ROWS, CONF_F])
    slabjunk = Tt("slabjunk", [ROWS, CONF_F])
    ldiff = Tt("ldiff", [ROWS, 68, 2])
    lsq = Tt("lsq", [ROWS, 68, 2])
    pairsum = Tt("pairsum", [ROWS, 68, 1])
    d_t = Tt("d_t", [ROWS, 68, 1])
    s_t = Tt("s_t", [ROWS, 1])
    bdiff = Tt("bdiff", [ROWS, 4])
    bneg = Tt("bneg", [ROWS, 4])
    bad = Tt("bad", [ROWS, 4])
    bt_ = Tt("bt_", [ROWS, 4])
    bth = Tt("bth", [ROWS, 4])
    bu = Tt("bu", [ROWS, 4])
    bsl = Tt("bsl", [ROWS, 4])
    ones4 = Tt("ones4", [ROWS, 4])
    negh4 = Tt("negh4", [ROWS, 4])
    zero4 = Tt("zero4", [ROWS, 4])
    cm1 = Tt("cm1", [ROWS, 1])
    cm1sq = Tt("cm1sq", [ROWS, 1])
    csq_ = Tt("csq_", [ROWS, 1])
    outtile = Tt("outtile", [ROWS, 8])

    lmp_v = small_t[:, 0:136]
    lmt_v = small_t[:, 136:272]
    bbp_v = small_t[:, 272:276]
    bbt_v = small_t[:, 276:280]
    aux0 = small_t[:, 280:281]   # gathered conf
    aux1 = small_t[:, 281:282]   # (mask / nf)^2  (folded into the ACT sqrt scale)
    aux2 = small_t[:, 282:283]   # mask
    conf_v = conf_t[:]                      # zero-padded dense conf channel

    op = mybir.AluOpType
    ax = mybir.AxisListType

    with nc.Block() as block, \
            nc.semaphore("dsem") as dsem, \
            nc.semaphore("vsem") as vsem, \
            nc.semaphore("csem") as csem, \
            nc.semaphore("msem") as msem, \
            nc.semaphore("osem") as osem:

        @block.sync
        def _(sync):
            sync.dma_start(out=small_t[:], in_=small_p[:]).then_inc(dsem, 16)
            sync.dma_start(out=conf_t[:], in_=conf_p[:]).then_inc(dsem, 16)
            sync.wait_ge(csem, 3)
            sync.dma_start(out=out_p[:], in_=outtile[:]).then_inc(osem, 16)
            sync.wait_ge(osem, 16)

        @block.vector
        def _(vector):
            vector.memset(outtile[:], 0.0)
            vector.memset(ones4[:], 1.0)
            vector.memset(negh4[:], -0.5)
            vector.memset(zero4[:], 0.0)
            vector.drain().then_inc(msem, 1)
            vector.wait_ge(dsem, 16)        # small rows landed
            # L1
            vector.tensor_tensor(out=ldiff[:], in0=lmp_v, in1=lmt_v, op=op.subtract)
            vector.tensor_tensor(out=bdiff[:], in0=bbp_v, in1=bbt_v, op=op.subtract)
            vector.tensor_tensor(out=cm1[:], in0=aux0, in1=ones4[:, 0:1], op=op.subtract)
            vector.tensor_tensor(out=csq_[:], in0=aux0, in1=aux0, op=op.mult)
            vector.drain()
            # L2
            vector.tensor_mul(lsq[:], ldiff[:], ldiff[:])
            vector.tensor_sub(bneg[:], zero4[:], bdiff[:])
            vector.tensor_mul(cm1sq[:], cm1[:], cm1[:])
            vector.tensor_mul(outtile[:, 4:5], csq_[:], aux2)
            vector.drain()
            # L3
            vector.tensor_tensor(out=pairsum[:], in0=lsq[:, :, 0:1], in1=lsq[:, :, 1:2], op=op.add)
            vector.tensor_tensor(out=bad[:], in0=bdiff[:], in1=bneg[:], op=op.max)
            vector.tensor_mul(outtile[:, 3:4], cm1sq[:], aux2)
            vector.drain().then_inc(vsem, 1)      # ACT may start sqrt
            # smooth-L1 tail: t=min(|d|,1); sl1 = t*(|d| - 0.5 t)
            vector.tensor_tensor(out=bt_[:], in0=bad[:], in1=ones4[:], op=op.min)
            vector.drain()
            vector.tensor_mul(bth[:], bt_[:], negh4[:])
            vector.drain()
            vector.tensor_add(bu[:], bad[:], bth[:])
            vector.drain()
            vector.tensor_mul(bsl[:], bt_[:], bu[:])
            vector.drain()
            vector.tensor_reduce(out=outtile[:, 2:3], in_=bsl[:], axis=ax.X, op=op.add)
            vector.drain().then_inc(csem, 1)

        @block.scalar
        def _(scalar):
            # dense conf^2 (host pre-extracted channel, zero-padded) — runs
            # on ACT in parallel with the DVE row pipeline, before the sqrt.
            scalar.wait_ge(msem, 1)         # outtile memset drained
            scalar.wait_ge(dsem, 32)
            scalar.activation(
                out=slabjunk[:], in_=conf_v,
                func=mybir.ActivationFunctionType.Square,
                accum_out=outtile[:, 0:1],
            )
            scalar.drain().then_inc(csem, 1)
            # weighted landmark distances in one op:
            # sqrt(pairsum * w^2) = w * sqrt(dx^2+dy^2);  accum -> nme partials
            scalar.wait_ge(vsem, 1)
            scalar.activation(
                out=d_t[:], in_=pairsum[:],
                func=mybir.ActivationFunctionType.Sqrt, scale=aux1,
                accum_out=outtile[:, 1:2],
            )
            scalar.drain().then_inc(csem, 1)

    st.close()
    return nc


def _get_nc():
    if "nc" not in _STATE:
        _STATE["nc"] = _build_program()
    return _STATE["nc"]


def _build_targets_host(bbox_target):
    """Replicate reference build_targets' cell assignment exactly (jax-CPU),
    returning the winning target index per grid cell (-1 = no object)."""
    import jax
    import jax.numpy as jnp

    cpu = jax.devices("cpu")[0]
    with jax.default_device(cpu):
        bt = jnp.asarray(np.asarray(bbox_target), dtype=jnp.float32)
        gt = bt[..., :4]
        valid = jnp.sum(bt, axis=-1) != 0
        gi = (gt[..., 0] * G).astype(jnp.int32)
        gj = (gt[..., 1] * G).astype(jnp.int32)
        acx = (0.5 + gi.astype(gt.dtype)) / G
        acy = (0.5 + gj.astype(gt.dtype)) / G
        aw = jnp.asarray(ANCHORS)[:, 0]
        ah = jnp.asarray(ANCHORS)[:, 1]

        def corners(cx, cy, w, h):
            x1 = (cx - w / 2) * IMAGE_SIZE
            x2 = (cx + w / 2) * IMAGE_SIZE
            y1 = (cy - h / 2) * IMAGE_SIZE
            y2 = (cy + h / 2) * IMAGE_SIZE
            return x1, x2, y1, y2

        gx1, gx2, gy1, gy2 = corners(gt[..., 0], gt[..., 1], gt[..., 2], gt[..., 3])
        ax1, ax2, ay1, ay2 = corners(acx[..., None], acy[..., None], aw, ah)
        ix1 = jnp.maximum(gx1[..., None], ax1)
        iy1 = jnp.maximum(gy1[..., None], ay1)
        ix2 = jnp.minimum(gx2[..., None], ax2)
        iy2 = jnp.minimum(gy2[..., None], ay2)
        inter = (ix2 - ix1 + 1) * (iy2 - iy1 + 1)
        area_g = ((gx2 - gx1 + 1) * (gy2 - gy1 + 1))[..., None]
        area_a = (ax2 - ax1 + 1) * (ay2 - ay1 + 1)
        iou = inter / (area_g + area_a - inter + 1e-16)
        best = jnp.argmax(iou, axis=-1)
        b_idx = jnp.broadcast_to(jnp.arange(B)[:, None], (B, T))
        gj_s = jnp.where(valid, gj, G)
        tnum = jnp.broadcast_to(jnp.arange(T)[None, :], (B, T))
        win = (
            jnp.full((B, G, G, A), -1, jnp.int32)
            .at[b_idx, gj_s, gi, best]
            .set(tnum, mode="drop")
        )
    return np.asarray(win)


def _prepare(bbox_prediction, landmarks_prediction, bbox_target, landmarks_target):
    """Host prep: target assignment + gather.  Returns (in_maps, n_obj)."""
    bbox_prediction = np.asarray(bbox_prediction, dtype=np.float32)
    landmarks_prediction = np.asarray(landmarks_prediction, dtype=np.float32)
    bbox_target = np.asarray(bbox_target, dtype=np.float32)
    landmarks_target = np.asarray(landmarks_target, dtype=np.float32)

    win = _build_targets_host(bbox_target)
    cells = np.argwhere(win >= 0)                      # (n, 4): b, gj, gi, a
    twin = win[win >= 0]                               # aligned winners
    n_obj = len(cells)

    cb, cj, ci, ca = cells[:, 0], cells[:, 1], cells[:, 2], cells[:, 3]
    lmp_all = landmarks_prediction[cb, cj, ci, ca].reshape(n_obj, 136)
    lmt_all = landmarks_target[cb, twin].reshape(n_obj, 136)
    bbp_all = bbox_prediction[cb, cj, ci, ca, :4]      # (n, 4)
    bbt_all = np.log1p(bbox_target[cb, twin, :4]).astype(np.float32)
    conf_all = bbox_prediction[cb, cj, ci, ca, 4]      # (n,)
    nf_all = np.sqrt(bbt_all[:, 2] * bbt_all[:, 3]).astype(np.float32)
    w_all = (np.float32(1.0) / nf_all).astype(np.float32)

    in_maps = []
    for c in range(NCORES):
        sel = (cb >= c * BPC) & (cb < (c + 1) * BPC)
        r = int(sel.sum())
        small = np.zeros((ROWS, 284), np.float32)
        small[:r, 0:136] = lmp_all[sel]
        small[:r, 136:272] = lmt_all[sel]
        small[:r, 272:276] = bbp_all[sel]
        small[:r, 276:280] = bbt_all[sel]
        small[:r, 280] = conf_all[sel]
        small[:r, 281] = (w_all * w_all)[sel]
        small[:r, 282] = 1.0
        confc = bbox_prediction[c * BPC:(c + 1) * BPC, :, :, :, 4].reshape(-1)
        conf_pad = np.zeros(ROWS * CONF_F, np.float32)
        conf_pad[:confc.size] = confc
        in_maps.append({"small": small, "conf": conf_pad.reshape(ROWS, CONF_F)})
    return in_maps, n_obj


def _combine(results, n_obj):
    S = np.zeros(5, np.float64)
    for r in results:
        o = r["out"].astype(np.float64)
        S += o[:, :5].sum(axis=0)
    s_slab, s_nme, s_loc, s_cse, s_csq = S
    n_obj_c = max(float(n_obj), 1.0)
    n_noobj = max(float(B * CELLS - n_obj), 1.0)
    nme = 2.0 * s_nme / (68.0 * n_obj_c)
    loc = 5.0 * s_loc / (n_obj_c * 4.0)
    conf = 0.5 * (s_slab - s_csq) / n_noobj + s_cse / n_obj_c
    return (np.float32(nme), np.float32(loc), np.float32(conf))


def _run_device(in_maps, trace=False):
    from concourse.bass_utils import run_bass_kernel_spmd
    nc = _get_nc()
    return run_bass_kernel_spmd(nc, in_maps, list(range(NCORES)), trace=trace)


def kernel(bbox_prediction, landmarks_prediction, bbox_target, landmarks_target):
    in_maps, n_obj = _prepare(
        bbox_prediction, landmarks_prediction, bbox_target, landmarks_target)
    res = _run_device(in_maps)
    return _combine(res.results, n_obj)



# revision 15
# speedup vs baseline: 1.3883x; 1.3883x over previous
"""JointLoss (YOLO-style bbox + landmarks + confidence) on 8 Trainium2 cores.

v2: same host/device split as the baseline (host does target assignment +
gather, device computes the three losses), restructured for latency:

- Inputs ship as bf16 (tolerance 2e-2; accumulations stay f32), merged into
  two DMAs: `rows` (gathered obj-cell rows, issued by SP) and `conf` (dense
  conf channel, issued by ACT in parallel -- HWDGE paths don't serialize).
- Landmarks are packed x-block|y-block so the pair-sum is a packed 2x-mode
  tensor_tensor instead of a strided add.
- Smooth-L1 uses the identity  sl1(d) = 0.5*d^2 - 0.5*relu(|d|-1)^2, with
  q = (|d| abs_max 1) - 1 computed in ONE fused tensor_scalar, so the whole
  bbox term is two small DVE ops squeezed into the shadow of the ACT sqrt.
- The critical DVE chain is only sub -> square -> pairadd (3 levels).
- Output DMA is a Pool-engine PREPARED kv_writeback: descriptors are
  generated during the input-DMA dead window; when the partials are ready a
  cheap trigger_dma fires it, skipping the ~1.3us HWDGE+DGE launch path.
- Per-partition partials [128,12] go back; host reduces in f64.
"""

import numpy as np
import ml_dtypes

BF16 = ml_dtypes.bfloat16

B, T, G, A = 32, 32, 36, 5
NCORES = 8
BPC = B // NCORES            # batches per core
CELLS = G * G * A            # 6480 per batch
ROWS = 128                   # padded obj rows per core (max B/NCORES*T)
CONF_N = BPC * CELLS         # 25920 dense conf elems per core
CONF_F = 204                 # 128*204 = 26112 >= CONF_N
# rows cols (bf16): a 0:142 | b 142:284 | w2(f32) 284:286 | pad | conf 288:492
ROWC = 288 + CONF_F
OUTC = 16                    # out cols: bd^2(4) cm1^2 cconf^2 r1^2(4) r2^2(4) nme slab

IMAGE_SIZE = 288.0
ANCHORS = np.array([[0.24, 0.24], [0.12, 0.12], [0.08, 0.08],
                    [0.28, 0.28], [0.15, 0.15]], dtype=np.float32)

_STATE = {}

# Output path: True = Pool-prepared kv_writeback + trigger_dma,
# False = plain SP HWDGE dma_start (fallback).
OUT_VIA_TRIGGER = True
# Attach no semaphore wait to the output DMA (NRT drains DMA queues at
# kernel end); False adds an SP-side wait on the completion sem.
NO_FINAL_WAIT = True


def _build_program():
    import concourse.bass as bass
    from concourse import mybir
    from concourse import library_config
    from contextlib import ExitStack

    nc = bass.Bass()
    f32 = mybir.dt.float32
    bf16 = mybir.dt.bfloat16
    i32 = mybir.dt.int32
    op = mybir.AluOpType
    fn = mybir.ActivationFunctionType

    rows_p = nc.declare_dram_parameter("rows", [ROWS, ROWC], bf16, isOutput=False)
    out_p = nc.declare_dram_parameter("out", [1, ROWS, 1, OUTC], f32, isOutput=True)

    st = ExitStack()
    Tt = lambda n, s, d: st.enter_context(nc.sbuf_tensor(n, s, d))
    rows_t = Tt("rows_t", [ROWS, ROWC], bf16)
    sub_t = Tt("sub_t", [ROWS, 150], bf16)   # ldx ldy | bd 136:140 | cm1 | cconf | r1 142:146 | r2 146:150
    sq_t = Tt("sq_t", [ROWS, 136], bf16)
    ps_t = Tt("ps_t", [ROWS, 68], bf16)
    dj_t = Tt("dj_t", [ROWS, 68], bf16)      # sqrt elementwise out (junk)
    cj_t = Tt("cj_t", [ROWS, CONF_F], bf16)  # conf^2 elementwise out (junk)
    out_t = Tt("out_t", [ROWS, 1, 1, OUTC], f32)
    ctx_t = Tt("ctx_t", [ROWS, 1], i32)

    def o2(a, b):            # 2-D [128, b-a] view of out_t cols
        return out_t[:, 0:1, 0:1, a:b].squeeze(1).squeeze(1)

    w2_ap = rows_t[:, 284:286].bitcast(f32)  # [128,1] f32 NME scale

    with nc.Block() as block, \
            nc.semaphore("dsa") as dsa, \
            nc.semaphore("vsem") as vsem, \
            nc.semaphore("csem") as csem, \
            nc.semaphore("psem") as psem, \
            nc.semaphore("osem") as osem:

        @block.sync
        def _(sync):
            sync.dma_start(out=rows_t[:], in_=rows_p[:]).then_inc(dsa, 16)
            if not OUT_VIA_TRIGGER:
                sync.wait_ge(csem, 2)
                sync.dma_start(out=out_p[:], in_=out_t[:]).then_inc(osem, 16)
                if not NO_FINAL_WAIT:
                    sync.wait_ge(osem, 16)
            elif not NO_FINAL_WAIT:
                sync.wait_ge(osem, 16)

        @block.scalar
        def _(scalar):
            scalar.wait_ge(dsa, 16)
            scalar.activation(out=cj_t[:], in_=rows_t[:, 288:288 + CONF_F],
                              func=fn.Square, accum_out=o2(15, 16))
            scalar.wait_ge(vsem, 1)
            scalar.activation(out=dj_t[:], in_=ps_t[:], func=fn.Sqrt,
                              scale=w2_ap, accum_out=o2(14, 15))
            scalar.drain().then_inc(csem, 1)

        @block.vector
        def _(vector):
            vector.wait_ge(dsa, 16)
            vector.tensor_tensor(out=sub_t[:, 0:142], in0=rows_t[:, 0:142],
                                 in1=rows_t[:, 142:284], op=op.subtract)
            vector.drain()
            vector.tensor_tensor(out=sq_t[:], in0=sub_t[:, 0:136],
                                 in1=sub_t[:, 0:136], op=op.mult)
            vector.drain()
            vector.tensor_tensor(out=ps_t[:], in0=sq_t[:, 0:68],
                                 in1=sq_t[:, 68:136], op=op.add)
            vector.drain().then_inc(vsem, 1)
            # sl1 = 0.5*d^2 - 0.5*relu(|d|-1)^2; relu(|d|-1)^2 = r1^2 + r2^2
            # with r1 = relu(d-1) = (d-1) max 0, r2 = min(d+1, 0).
            vector.tensor_scalar(out=sub_t[:, 142:146], in0=sub_t[:, 136:140],
                                 scalar1=1.0, scalar2=0.0,
                                 op0=op.subtract, op1=op.max)
            vector.tensor_scalar(out=sub_t[:, 146:150], in0=sub_t[:, 136:140],
                                 scalar1=1.0, scalar2=0.0,
                                 op0=op.add, op1=op.min)
            vector.drain()
            vector.tensor_tensor(out=o2(0, 14), in0=sub_t[:, 136:150],
                                 in1=sub_t[:, 136:150], op=op.mult)
            vector.drain().then_inc(csem, 1)

        if OUT_VIA_TRIGGER:
            @block.gpsimd
            def _(gpsimd):
                gpsimd.load_library(library_config.attn)
                gpsimd.memset(ctx_t[:], 0)
                gpsimd.kv_writeback(
                    out_ap=out_p[:], in_ap=out_t[:], ctx_idxs_ap=ctx_t[:],
                    prepare_only=True, sem=osem,
                ).then_inc(psem, 1)
                gpsimd.wait_ge(psem, 1)
                gpsimd.wait_ge(csem, 2)
                trig = gpsimd.trigger_dma(count=1)
                # bass_rust hardcodes the older opcode numbering (235 = this
                # toolchain's HINT); rewrite to this ISA's TRIGGER_DMA.
                trig.ins.isa_opcode = int(
                    nc.isa.Opcode.NEURON_ISA_TPB_OPCODE_TRIGGER_DMA.value)

    st.close()
    # Raw Bass skips Bacc's ISA-subclass lowering; run it so the trigger /
    # library-reload pseudo instructions get real ISA bytes for walrus.
    mybir.codegen_inst_isa_subclasses(nc)
    return nc


def _get_nc():
    if "nc" not in _STATE:
        _STATE["nc"] = _build_program()
    return _STATE["nc"]


def _build_targets_host(bbox_target):
    """Replicate reference build_targets' cell assignment exactly (jax-CPU),
    returning the winning target index per grid cell (-1 = no object)."""
    import jax
    import jax.numpy as jnp

    cpu = jax.devices("cpu")[0]
    with jax.default_device(cpu):
        bt = jnp.asarray(np.asarray(bbox_target), dtype=jnp.float32)
        gt = bt[..., :4]
        valid = jnp.sum(bt, axis=-1) != 0
        gi = (gt[..., 0] * G).astype(jnp.int32)
        gj = (gt[..., 1] * G).astype(jnp.int32)
        acx = (0.5 + gi.astype(gt.dtype)) / G
        acy = (0.5 + gj.astype(gt.dtype)) / G
        aw = jnp.asarray(ANCHORS)[:, 0]
        ah = jnp.asarray(ANCHORS)[:, 1]

        def corners(cx, cy, w, h):
            x1 = (cx - w / 2) * IMAGE_SIZE
            x2 = (cx + w / 2) * IMAGE_SIZE
            y1 = (cy - h / 2) * IMAGE_SIZE
            y2 = (cy + h / 2) * IMAGE_SIZE
            return x1, x2, y1, y2

        gx1, gx2, gy1, gy2 = corners(gt[..., 0], gt[..., 1], gt[..., 2], gt[..., 3])
        ax1, ax2, ay1, ay2 = corners(acx[..., None], acy[..., None], aw, ah)
        ix1 = jnp.maximum(gx1[..., None], ax1)
        iy1 = jnp.maximum(gy1[..., None], ay1)
        ix2 = jnp.minimum(gx2[..., None], ax2)
        iy2 = jnp.minimum(gy2[..., None], ay2)
        inter = (ix2 - ix1 + 1) * (iy2 - iy1 + 1)
        area_g = ((gx2 - gx1 + 1) * (gy2 - gy1 + 1))[..., None]
        area_a = (ax2 - ax1 + 1) * (ay2 - ay1 + 1)
        iou = inter / (area_g + area_a - inter + 1e-16)
        best = jnp.argmax(iou, axis=-1)
        b_idx = jnp.broadcast_to(jnp.arange(B)[:, None], (B, T))
        gj_s = jnp.where(valid, gj, G)
        tnum = jnp.broadcast_to(jnp.arange(T)[None, :], (B, T))
        win = (
            jnp.full((B, G, G, A), -1, jnp.int32)
            .at[b_idx, gj_s, gi, best]
            .set(tnum, mode="drop")
        )
    return np.asarray(win)


def _prepare(bbox_prediction, landmarks_prediction, bbox_target, landmarks_target):
    """Host prep: target assignment + gather + bf16 packing."""
    bbox_prediction = np.asarray(bbox_prediction, dtype=np.float32)
    landmarks_prediction = np.asarray(landmarks_prediction, dtype=np.float32)
    bbox_target = np.asarray(bbox_target, dtype=np.float32)
    landmarks_target = np.asarray(landmarks_target, dtype=np.float32)

    win = _build_targets_host(bbox_target)
    cells = np.argwhere(win >= 0)                      # (n, 4): b, gj, gi, a
    twin = win[win >= 0]                               # aligned winners
    n_obj = len(cells)

    cb, cj, ci, ca = cells[:, 0], cells[:, 1], cells[:, 2], cells[:, 3]
    lmp_all = landmarks_prediction[cb, cj, ci, ca]     # (n, 68, 2)
    lmt_all = landmarks_target[cb, twin]               # (n, 68, 2)
    bbp_all = bbox_prediction[cb, cj, ci, ca, :4]      # (n, 4)
    bbt_all = np.log1p(bbox_target[cb, twin, :4]).astype(np.float32)
    conf_all = bbox_prediction[cb, cj, ci, ca, 4]      # (n,)
    w2_all = (np.float32(1.0) / (bbt_all[:, 2] * bbt_all[:, 3])).astype(np.float32)

    in_maps = []
    for c in range(NCORES):
        sel = (cb >= c * BPC) & (cb < (c + 1) * BPC)
        r = int(sel.sum())
        rows = np.zeros((ROWS, ROWC), BF16)
        rows[:r, 0:68] = lmp_all[sel][:, :, 0]
        rows[:r, 68:136] = lmp_all[sel][:, :, 1]
        rows[:r, 136:140] = bbp_all[sel]
        rows[:r, 140] = conf_all[sel]
        rows[:r, 141] = conf_all[sel]
        rows[:r, 142:210] = lmt_all[sel][:, :, 0]
        rows[:r, 210:278] = lmt_all[sel][:, :, 1]
        rows[:r, 278:282] = bbt_all[sel]
        rows[:r, 282] = 1.0
        w2c = np.zeros(ROWS, np.float32)
        w2c[:r] = w2_all[sel]
        rows.view(np.uint16)[:, 284:286] = w2c.view(np.uint16).reshape(ROWS, 2)
        confc = bbox_prediction[c * BPC:(c + 1) * BPC, :, :, :, 4].reshape(-1)
        conf_pad = np.zeros(ROWS * CONF_F, BF16)
        conf_pad[:CONF_N] = confc
        rows[:, 288:288 + CONF_F] = conf_pad.reshape(ROWS, CONF_F)
        in_maps.append({"rows": rows})
    return in_maps, n_obj


def _combine(results, n_obj):
    S = np.zeros(OUTC, np.float64)
    for r in results:
        S += r["out"].reshape(ROWS, OUTC).astype(np.float64).sum(axis=0)
    s_bsq = S[0:4].sum()
    s_cse = S[4]
    s_csq = S[5]
    s_q = S[6:14].sum()
    s_nme = S[14]
    s_slab = S[15]
    n = max(float(n_obj), 1.0)
    n_noobj = max(float(B * CELLS - n_obj), 1.0)
    nme = 2.0 * s_nme / (68.0 * n)
    loc = 5.0 * 0.5 * (s_bsq - s_q) / (n * 4.0)
    conf = 0.5 * (s_slab - s_csq) / n_noobj + s_cse / n
    return (np.float32(nme), np.float32(loc), np.float32(conf))


def _run_device(in_maps, trace=False):
    from concourse.bass_utils import run_bass_kernel_spmd
    nc = _get_nc()
    return run_bass_kernel_spmd(nc, in_maps, list(range(NCORES)), trace=trace)


def kernel(bbox_prediction, landmarks_prediction, bbox_target, landmarks_target):
    in_maps, n_obj = _prepare(
        bbox_prediction, landmarks_prediction, bbox_target, landmarks_target)
    res = _run_device(in_maps)
    return _combine(res.results, n_obj)


# revision 17
# speedup vs baseline: 1.4525x; 1.0462x over previous
"""JointLoss (YOLO-style bbox + landmarks + confidence) on 8 Trainium2 cores.

v2: same host/device split as the baseline (host does target assignment +
gather, device computes the three losses), restructured for latency:

- Inputs ship as bf16 (tolerance 2e-2; accumulations stay f32), merged into
  two DMAs: `rows` (gathered obj-cell rows, issued by SP) and `conf` (dense
  conf channel, issued by ACT in parallel -- HWDGE paths don't serialize).
- Landmarks are packed x-block|y-block so the pair-sum is a packed 2x-mode
  tensor_tensor instead of a strided add.
- Smooth-L1 uses the identity  sl1(d) = 0.5*d^2 - 0.5*relu(|d|-1)^2, with
  q = (|d| abs_max 1) - 1 computed in ONE fused tensor_scalar, so the whole
  bbox term is two small DVE ops squeezed into the shadow of the ACT sqrt.
- The critical DVE chain is only sub -> square -> pairadd (3 levels).
- Output DMA is a Pool-engine PREPARED kv_writeback: descriptors are
  generated during the input-DMA dead window; when the partials are ready a
  cheap trigger_dma fires it, skipping the ~1.3us HWDGE+DGE launch path.
- Per-partition partials [128,12] go back; host reduces in f64.
"""

import numpy as np
import ml_dtypes

BF16 = ml_dtypes.bfloat16

B, T, G, A = 32, 32, 36, 5
NCORES = 8
BPC = B // NCORES            # batches per core
CELLS = G * G * A            # 6480 per batch
ROWS = 128                   # padded obj rows per core (max B/NCORES*T)
CONF_N = BPC * CELLS         # 25920 dense conf elems per core
CONF_F = 204                 # 128*204 = 26112 >= CONF_N
# rows cols (bf16): a 0:142 | b 142:284 | w2(f32) 284:286 | pad | conf 288:492
ROWC = 288 + CONF_F
OUTC = 16                    # out cols: bd^2(4) cm1^2 cconf^2 r1^2(4) r2^2(4) nme slab

IMAGE_SIZE = 288.0
ANCHORS = np.array([[0.24, 0.24], [0.12, 0.12], [0.08, 0.08],
                    [0.28, 0.28], [0.15, 0.15]], dtype=np.float32)

_STATE = {}

# Output path: True = Pool-prepared kv_writeback + trigger_dma,
# False = plain SP HWDGE dma_start (fallback).
OUT_VIA_TRIGGER = True
# Attach no semaphore wait to the output DMA (NRT drains DMA queues at
# kernel end); False adds an SP-side wait on the completion sem.
NO_FINAL_WAIT = True


def _build_program():
    import concourse.bass as bass
    from concourse import mybir
    from concourse import library_config
    from contextlib import ExitStack

    nc = bass.Bass()
    f32 = mybir.dt.float32
    bf16 = mybir.dt.bfloat16
    i32 = mybir.dt.int32
    op = mybir.AluOpType
    fn = mybir.ActivationFunctionType

    # Drop the framework's const-AP memsets we never read (only f32-0.0 is
    # used, as the implicit activation bias); they serialize on Pool ahead
    # of the initial all-engine barrier.
    _unused = {"const-float32-1.0", "const-bfloat16-1.0", "const-uint8-127"}
    for _bb in nc.m.functions[0].blocks:
        _keep = []
        for _ins in _bb.instructions:
            _is_unused_memset = (
                str(_ins.opcode) in ("Memset", "InstructionName.Memset")
                and _ins.outs
                and any(u in str(_ins.outs[0]) for u in _unused)
            )
            if _is_unused_memset:
                del nc.inst_map[_ins.name]
            else:
                _keep.append(_ins)
        if len(_keep) != len(_bb.instructions):
            _bb.instructions[:] = _keep

    rows_p = nc.declare_dram_parameter("rows", [ROWS, ROWC], bf16, isOutput=False)
    out_p = nc.declare_dram_parameter("out", [1, ROWS, 1, OUTC], f32, isOutput=True)

    st = ExitStack()
    Tt = lambda n, s, d: st.enter_context(nc.sbuf_tensor(n, s, d))
    rows_t = Tt("rows_t", [ROWS, ROWC], bf16)
    sub_t = Tt("sub_t", [ROWS, 150], bf16)   # ldx ldy | bd 136:140 | cm1 | cconf | r1 142:146 | r2 146:150
    sq_t = Tt("sq_t", [ROWS, 136], bf16)
    ps_t = Tt("ps_t", [ROWS, 68], bf16)
    dj_t = Tt("dj_t", [ROWS, 68], bf16)      # sqrt elementwise out (junk)
    cj_t = Tt("cj_t", [ROWS, CONF_F], bf16)  # conf^2 elementwise out (junk)
    out_t = Tt("out_t", [ROWS, 1, 1, OUTC], f32)
    ctx_t = Tt("ctx_t", [ROWS, 1], i32)

    def o2(a, b):            # 2-D [128, b-a] view of out_t cols
        return out_t[:, 0:1, 0:1, a:b].squeeze(1).squeeze(1)

    w2_ap = rows_t[:, 284:286].bitcast(f32)  # [128,1] f32 NME scale

    with nc.Block() as block, \
            nc.semaphore("dsa") as dsa, \
            nc.semaphore("vsem") as vsem, \
            nc.semaphore("csem") as csem, \
            nc.semaphore("psem") as psem, \
            nc.semaphore("osem") as osem:

        @block.sync
        def _(sync):
            sync.dma_start(out=rows_t[:], in_=rows_p[:]).then_inc(dsa, 16)
            if not OUT_VIA_TRIGGER:
                sync.wait_ge(csem, 2)
                sync.dma_start(out=out_p[:], in_=out_t[:]).then_inc(osem, 16)
                if not NO_FINAL_WAIT:
                    sync.wait_ge(osem, 16)
            elif not NO_FINAL_WAIT:
                sync.wait_ge(osem, 16)

        @block.scalar
        def _(scalar):
            scalar.wait_ge(dsa, 16)
            scalar.activation(out=cj_t[:], in_=rows_t[:, 288:288 + CONF_F],
                              func=fn.Square, accum_out=o2(15, 16))
            scalar.wait_ge(vsem, 1)
            scalar.activation(out=dj_t[:], in_=ps_t[:], func=fn.Sqrt,
                              scale=w2_ap, accum_out=o2(14, 15))
            scalar.drain().then_inc(csem, 1)

        @block.vector
        def _(vector):
            vector.wait_ge(dsa, 16)
            vector.tensor_tensor(out=sub_t[:, 0:142], in0=rows_t[:, 0:142],
                                 in1=rows_t[:, 142:284], op=op.subtract)
            vector.drain()
            vector.tensor_tensor(out=sq_t[:], in0=sub_t[:, 0:136],
                                 in1=sub_t[:, 0:136], op=op.mult)
            vector.drain()
            vector.tensor_tensor(out=ps_t[:], in0=sq_t[:, 0:68],
                                 in1=sq_t[:, 68:136], op=op.add)
            vector.drain().then_inc(vsem, 1)
            # sl1 = 0.5*d^2 - 0.5*relu(|d|-1)^2; relu(|d|-1)^2 = r1^2 + r2^2
            # with r1 = relu(d-1) = (d-1) max 0, r2 = min(d+1, 0).
            vector.tensor_scalar(out=sub_t[:, 142:146], in0=sub_t[:, 136:140],
                                 scalar1=1.0, scalar2=0.0,
                                 op0=op.subtract, op1=op.max)
            vector.tensor_scalar(out=sub_t[:, 146:150], in0=sub_t[:, 136:140],
                                 scalar1=1.0, scalar2=0.0,
                                 op0=op.add, op1=op.min)
            vector.drain()
            vector.tensor_tensor(out=o2(0, 14), in0=sub_t[:, 136:150],
                                 in1=sub_t[:, 136:150], op=op.mult)
            vector.drain().then_inc(csem, 1)

        if OUT_VIA_TRIGGER:
            @block.gpsimd
            def _(gpsimd):
                gpsimd.load_library(library_config.attn)
                gpsimd.memset(ctx_t[:], 0)
                gpsimd.kv_writeback(
                    out_ap=out_p[:], in_ap=out_t[:], ctx_idxs_ap=ctx_t[:],
                    prepare_only=True, sem=osem,
                ).then_inc(psem, 1)
                gpsimd.wait_ge(psem, 1)
                gpsimd.wait_ge(csem, 2)
                trig = gpsimd.trigger_dma(count=1)
                # bass_rust hardcodes the older opcode numbering (235 = this
                # toolchain's HINT); rewrite to this ISA's TRIGGER_DMA.
                trig.ins.isa_opcode = int(
                    nc.isa.Opcode.NEURON_ISA_TPB_OPCODE_TRIGGER_DMA.value)

    st.close()
    # Raw Bass skips Bacc's ISA-subclass lowering; run it so the trigger /
    # library-reload pseudo instructions get real ISA bytes for walrus.
    mybir.codegen_inst_isa_subclasses(nc)
    return nc


def _get_nc():
    if "nc" not in _STATE:
        _STATE["nc"] = _build_program()
    return _STATE["nc"]


def _build_targets_host(bbox_target):
    """Replicate reference build_targets' cell assignment exactly (jax-CPU),
    returning the winning target index per grid cell (-1 = no object)."""
    import jax
    import jax.numpy as jnp

    cpu = jax.devices("cpu")[0]
    with jax.default_device(cpu):
        bt = jnp.asarray(np.asarray(bbox_target), dtype=jnp.float32)
        gt = bt[..., :4]
        valid = jnp.sum(bt, axis=-1) != 0
        gi = (gt[..., 0] * G).astype(jnp.int32)
        gj = (gt[..., 1] * G).astype(jnp.int32)
        acx = (0.5 + gi.astype(gt.dtype)) / G
        acy = (0.5 + gj.astype(gt.dtype)) / G
        aw = jnp.asarray(ANCHORS)[:, 0]
        ah = jnp.asarray(ANCHORS)[:, 1]

        def corners(cx, cy, w, h):
            x1 = (cx - w / 2) * IMAGE_SIZE
            x2 = (cx + w / 2) * IMAGE_SIZE
            y1 = (cy - h / 2) * IMAGE_SIZE
            y2 = (cy + h / 2) * IMAGE_SIZE
            return x1, x2, y1, y2

        gx1, gx2, gy1, gy2 = corners(gt[..., 0], gt[..., 1], gt[..., 2], gt[..., 3])
        ax1, ax2, ay1, ay2 = corners(acx[..., None], acy[..., None], aw, ah)
        ix1 = jnp.maximum(gx1[..., None], ax1)
        iy1 = jnp.maximum(gy1[..., None], ay1)
        ix2 = jnp.minimum(gx2[..., None], ax2)
        iy2 = jnp.minimum(gy2[..., None], ay2)
        inter = (ix2 - ix1 + 1) * (iy2 - iy1 + 1)
        area_g = ((gx2 - gx1 + 1) * (gy2 - gy1 + 1))[..., None]
        area_a = (ax2 - ax1 + 1) * (ay2 - ay1 + 1)
        iou = inter / (area_g + area_a - inter + 1e-16)
        best = jnp.argmax(iou, axis=-1)
        b_idx = jnp.broadcast_to(jnp.arange(B)[:, None], (B, T))
        gj_s = jnp.where(valid, gj, G)
        tnum = jnp.broadcast_to(jnp.arange(T)[None, :], (B, T))
        win = (
            jnp.full((B, G, G, A), -1, jnp.int32)
            .at[b_idx, gj_s, gi, best]
            .set(tnum, mode="drop")
        )
    return np.asarray(win)


def _prepare(bbox_prediction, landmarks_prediction, bbox_target, landmarks_target):
    """Host prep: target assignment + gather + bf16 packing."""
    bbox_prediction = np.asarray(bbox_prediction, dtype=np.float32)
    landmarks_prediction = np.asarray(landmarks_prediction, dtype=np.float32)
    bbox_target = np.asarray(bbox_target, dtype=np.float32)
    landmarks_target = np.asarray(landmarks_target, dtype=np.float32)

    win = _build_targets_host(bbox_target)
    cells = np.argwhere(win >= 0)                      # (n, 4): b, gj, gi, a
    twin = win[win >= 0]                               # aligned winners
    n_obj = len(cells)

    cb, cj, ci, ca = cells[:, 0], cells[:, 1], cells[:, 2], cells[:, 3]
    lmp_all = landmarks_prediction[cb, cj, ci, ca]     # (n, 68, 2)
    lmt_all = landmarks_target[cb, twin]               # (n, 68, 2)
    bbp_all = bbox_prediction[cb, cj, ci, ca, :4]      # (n, 4)
    bbt_all = np.log1p(bbox_target[cb, twin, :4]).astype(np.float32)
    conf_all = bbox_prediction[cb, cj, ci, ca, 4]      # (n,)
    w2_all = (np.float32(1.0) / (bbt_all[:, 2] * bbt_all[:, 3])).astype(np.float32)

    in_maps = []
    for c in range(NCORES):
        sel = (cb >= c * BPC) & (cb < (c + 1) * BPC)
        r = int(sel.sum())
        rows = np.zeros((ROWS, ROWC), BF16)
        rows[:r, 0:68] = lmp_all[sel][:, :, 0]
        rows[:r, 68:136] = lmp_all[sel][:, :, 1]
        rows[:r, 136:140] = bbp_all[sel]
        rows[:r, 140] = conf_all[sel]
        rows[:r, 141] = conf_all[sel]
        rows[:r, 142:210] = lmt_all[sel][:, :, 0]
        rows[:r, 210:278] = lmt_all[sel][:, :, 1]
        rows[:r, 278:282] = bbt_all[sel]
        rows[:r, 282] = 1.0
        w2c = np.zeros(ROWS, np.float32)
        w2c[:r] = w2_all[sel]
        rows.view(np.uint16)[:, 284:286] = w2c.view(np.uint16).reshape(ROWS, 2)
        confc = bbox_prediction[c * BPC:(c + 1) * BPC, :, :, :, 4].reshape(-1)
        conf_pad = np.zeros(ROWS * CONF_F, BF16)
        conf_pad[:CONF_N] = confc
        rows[:, 288:288 + CONF_F] = conf_pad.reshape(ROWS, CONF_F)
        in_maps.append({"rows": rows})
    return in_maps, n_obj


def _combine(results, n_obj):
    S = np.zeros(OUTC, np.float64)
    for r in results:
        S += r["out"].reshape(ROWS, OUTC).astype(np.float64).sum(axis=0)
    s_bsq = S[0:4].sum()
    s_cse = S[4]
    s_csq = S[5]
    s_q = S[6:14].sum()
    s_nme = S[14]
    s_slab = S[15]
    n = max(float(n_obj), 1.0)
    n_noobj = max(float(B * CELLS - n_obj), 1.0)
    nme = 2.0 * s_nme / (68.0 * n)
    loc = 5.0 * 0.5 * (s_bsq - s_q) / (n * 4.0)
    conf = 0.5 * (s_slab - s_csq) / n_noobj + s_cse / n
    return (np.float32(nme), np.float32(loc), np.float32(conf))


def _run_device(in_maps, trace=False):
    from concourse.bass_utils import run_bass_kernel_spmd
    nc = _get_nc()
    return run_bass_kernel_spmd(nc, in_maps, list(range(NCORES)), trace=trace)


def kernel(bbox_prediction, landmarks_prediction, bbox_target, landmarks_target):
    in_maps, n_obj = _prepare(
        bbox_prediction, landmarks_prediction, bbox_target, landmarks_target)
    res = _run_device(in_maps)
    return _combine(res.results, n_obj)


# revision 18
# speedup vs baseline: 1.4992x; 1.0322x over previous
"""JointLoss (YOLO-style bbox + landmarks + confidence) on 8 Trainium2 cores.

v2: same host/device split as the baseline (host does target assignment +
gather, device computes the three losses), restructured for latency:

- Inputs ship as bf16 (tolerance 2e-2; accumulations stay f32), merged into
  two DMAs: `rows` (gathered obj-cell rows, issued by SP) and `conf` (dense
  conf channel, issued by ACT in parallel -- HWDGE paths don't serialize).
- Landmarks are packed x-block|y-block so the pair-sum is a packed 2x-mode
  tensor_tensor instead of a strided add.
- Smooth-L1 uses the identity  sl1(d) = 0.5*d^2 - 0.5*relu(|d|-1)^2, with
  q = (|d| abs_max 1) - 1 computed in ONE fused tensor_scalar, so the whole
  bbox term is two small DVE ops squeezed into the shadow of the ACT sqrt.
- The critical DVE chain is only sub -> square -> pairadd (3 levels).
- Output DMA is a Pool-engine PREPARED kv_writeback: descriptors are
  generated during the input-DMA dead window; when the partials are ready a
  cheap trigger_dma fires it, skipping the ~1.3us HWDGE+DGE launch path.
- Per-partition partials [128,12] go back; host reduces in f64.
"""

import numpy as np
import ml_dtypes

BF16 = ml_dtypes.bfloat16

B, T, G, A = 32, 32, 36, 5
NCORES = 8
BPC = B // NCORES            # batches per core
CELLS = G * G * A            # 6480 per batch
ROWS = 128                   # padded obj rows per core (max B/NCORES*T)
CONF_N = BPC * CELLS         # 25920 dense conf elems per core
CONF_F = 204                 # 128*204 = 26112 >= CONF_N
# rows cols (bf16): a 0:142 | b 142:284 | w2(f32) 284:286 | pad | conf 288:492
ROWC = 288 + CONF_F
OUTC = 16                    # out cols: bd^2(4) cm1^2 cconf^2 r1^2(4) r2^2(4) nme slab

IMAGE_SIZE = 288.0
ANCHORS = np.array([[0.24, 0.24], [0.12, 0.12], [0.08, 0.08],
                    [0.28, 0.28], [0.15, 0.15]], dtype=np.float32)

_STATE = {}

# Output path: True = Pool-prepared kv_writeback + trigger_dma,
# False = plain SP HWDGE dma_start (fallback).
OUT_VIA_TRIGGER = True
# Attach no semaphore wait to the output DMA (NRT drains DMA queues at
# kernel end); False adds an SP-side wait on the completion sem.
NO_FINAL_WAIT = True


def _build_program():
    import concourse.bass as bass
    from concourse import mybir
    from concourse import library_config
    from contextlib import ExitStack

    nc = bass.Bass()
    f32 = mybir.dt.float32
    bf16 = mybir.dt.bfloat16
    i32 = mybir.dt.int32
    op = mybir.AluOpType
    fn = mybir.ActivationFunctionType

    # Drop the framework's const-AP memsets we never read (only f32-0.0 is
    # used, as the implicit activation bias); they serialize on Pool ahead
    # of the initial all-engine barrier.
    _unused = {"const-float32-1.0", "const-bfloat16-1.0", "const-uint8-127"}
    for _bb in nc.m.functions[0].blocks:
        _keep = []
        for _ins in _bb.instructions:
            _is_unused_memset = (
                str(_ins.opcode) in ("Memset", "InstructionName.Memset")
                and _ins.outs
                and any(u in str(_ins.outs[0]) for u in _unused)
            )
            if _is_unused_memset:
                del nc.inst_map[_ins.name]
            else:
                _keep.append(_ins)
        if len(_keep) != len(_bb.instructions):
            _bb.instructions[:] = _keep

    rows_p = nc.declare_dram_parameter("rows", [ROWS, ROWC], bf16, isOutput=False)
    out_p = nc.declare_dram_parameter("out", [1, ROWS, 1, OUTC], f32, isOutput=True)

    st = ExitStack()
    Tt = lambda n, s, d: st.enter_context(nc.sbuf_tensor(n, s, d))
    rows_t = Tt("rows_t", [ROWS, ROWC], bf16)
    sub_t = Tt("sub_t", [ROWS, 150], bf16)   # ldx ldy | bd 136:140 | cm1 | cconf | r1 142:146 | r2 146:150
    sq_t = Tt("sq_t", [ROWS, 136], bf16)
    ps_t = Tt("ps_t", [ROWS, 68], bf16)
    dj_t = Tt("dj_t", [ROWS, 68], bf16)      # sqrt elementwise out (junk)
    cj_t = Tt("cj_t", [ROWS, CONF_F], bf16)  # conf^2 elementwise out (junk)
    out_t = Tt("out_t", [ROWS, 1, 1, OUTC], f32)
    ctx_t = Tt("ctx_t", [ROWS, 1], i32)

    def o2(a, b):            # 2-D [128, b-a] view of out_t cols
        return out_t[:, 0:1, 0:1, a:b].squeeze(1).squeeze(1)

    w2_ap = rows_t[:, 284:286].bitcast(f32)  # [128,1] f32 NME scale

    with nc.Block() as block, \
            nc.semaphore("dsa") as dsa, \
            nc.semaphore("vsem") as vsem, \
            nc.semaphore("csem") as csem, \
            nc.semaphore("psem") as psem, \
            nc.semaphore("osem") as osem:

        @block.sync
        def _(sync):
            sync.dma_start(out=rows_t[:], in_=rows_p[:]).then_inc(dsa, 16)
            if not OUT_VIA_TRIGGER:
                sync.wait_ge(csem, 2)
                sync.dma_start(out=out_p[:], in_=out_t[:]).then_inc(osem, 16)
                if not NO_FINAL_WAIT:
                    sync.wait_ge(osem, 16)
            elif not NO_FINAL_WAIT:
                sync.wait_ge(osem, 16)

        @block.scalar
        def _(scalar):
            scalar.activation(out=cj_t[:], in_=rows_t[:, 288:288 + CONF_F],
                              func=fn.Square, accum_out=o2(15, 16),
                              )._wait_ge(dsa, 16)
            scalar.activation(out=dj_t[:], in_=ps_t[:], func=fn.Sqrt,
                              scale=w2_ap, accum_out=o2(14, 15),
                              )._wait_ge(vsem, 1)
            scalar.drain().then_inc(csem, 1)

        @block.vector
        def _(vector):
            vector.tensor_tensor(out=sub_t[:, 0:142], in0=rows_t[:, 0:142],
                                 in1=rows_t[:, 142:284], op=op.subtract,
                                 )._wait_ge(dsa, 16)
            vector.drain()
            vector.tensor_tensor(out=sq_t[:], in0=sub_t[:, 0:136],
                                 in1=sub_t[:, 0:136], op=op.mult)
            vector.drain()
            vector.tensor_tensor(out=ps_t[:], in0=sq_t[:, 0:68],
                                 in1=sq_t[:, 68:136], op=op.add)
            vector.drain().then_inc(vsem, 1)
            # sl1 = 0.5*d^2 - 0.5*relu(|d|-1)^2; relu(|d|-1)^2 = r1^2 + r2^2
            # with r1 = relu(d-1) = (d-1) max 0, r2 = min(d+1, 0).
            vector.tensor_scalar(out=sub_t[:, 142:146], in0=sub_t[:, 136:140],
                                 scalar1=1.0, scalar2=0.0,
                                 op0=op.subtract, op1=op.max)
            vector.tensor_scalar(out=sub_t[:, 146:150], in0=sub_t[:, 136:140],
                                 scalar1=1.0, scalar2=0.0,
                                 op0=op.add, op1=op.min)
            vector.drain()
            vector.tensor_tensor(out=o2(0, 14), in0=sub_t[:, 136:150],
                                 in1=sub_t[:, 136:150], op=op.mult)
            vector.drain().then_inc(csem, 1)

        if OUT_VIA_TRIGGER:
            @block.gpsimd
            def _(gpsimd):
                gpsimd.load_library(library_config.attn)
                gpsimd.memset(ctx_t[:], 0)
                gpsimd.kv_writeback(
                    out_ap=out_p[:], in_ap=out_t[:], ctx_idxs_ap=ctx_t[:],
                    prepare_only=True, sem=osem,
                ).then_inc(psem, 1)
                gpsimd.wait_ge(psem, 1)
                gpsimd.wait_ge(csem, 2)
                trig = gpsimd.trigger_dma(count=1)
                # bass_rust hardcodes the older opcode numbering (235 = this
                # toolchain's HINT); rewrite to this ISA's TRIGGER_DMA.
                trig.ins.isa_opcode = int(
                    nc.isa.Opcode.NEURON_ISA_TPB_OPCODE_TRIGGER_DMA.value)

    st.close()
    # Raw Bass skips Bacc's ISA-subclass lowering; run it so the trigger /
    # library-reload pseudo instructions get real ISA bytes for walrus.
    mybir.codegen_inst_isa_subclasses(nc)
    return nc


def _get_nc():
    if "nc" not in _STATE:
        _STATE["nc"] = _build_program()
    return _STATE["nc"]


def _build_targets_host(bbox_target):
    """Replicate reference build_targets' cell assignment exactly (jax-CPU),
    returning the winning target index per grid cell (-1 = no object)."""
    import jax
    import jax.numpy as jnp

    cpu = jax.devices("cpu")[0]
    with jax.default_device(cpu):
        bt = jnp.asarray(np.asarray(bbox_target), dtype=jnp.float32)
        gt = bt[..., :4]
        valid = jnp.sum(bt, axis=-1) != 0
        gi = (gt[..., 0] * G).astype(jnp.int32)
        gj = (gt[..., 1] * G).astype(jnp.int32)
        acx = (0.5 + gi.astype(gt.dtype)) / G
        acy = (0.5 + gj.astype(gt.dtype)) / G
        aw = jnp.asarray(ANCHORS)[:, 0]
        ah = jnp.asarray(ANCHORS)[:, 1]

        def corners(cx, cy, w, h):
            x1 = (cx - w / 2) * IMAGE_SIZE
            x2 = (cx + w / 2) * IMAGE_SIZE
            y1 = (cy - h / 2) * IMAGE_SIZE
            y2 = (cy + h / 2) * IMAGE_SIZE
            return x1, x2, y1, y2

        gx1, gx2, gy1, gy2 = corners(gt[..., 0], gt[..., 1], gt[..., 2], gt[..., 3])
        ax1, ax2, ay1, ay2 = corners(acx[..., None], acy[..., None], aw, ah)
        ix1 = jnp.maximum(gx1[..., None], ax1)
        iy1 = jnp.maximum(gy1[..., None], ay1)
        ix2 = jnp.minimum(gx2[..., None], ax2)
        iy2 = jnp.minimum(gy2[..., None], ay2)
        inter = (ix2 - ix1 + 1) * (iy2 - iy1 + 1)
        area_g = ((gx2 - gx1 + 1) * (gy2 - gy1 + 1))[..., None]
        area_a = (ax2 - ax1 + 1) * (ay2 - ay1 + 1)
        iou = inter / (area_g + area_a - inter + 1e-16)
        best = jnp.argmax(iou, axis=-1)
        b_idx = jnp.broadcast_to(jnp.arange(B)[:, None], (B, T))
        gj_s = jnp.where(valid, gj, G)
        tnum = jnp.broadcast_to(jnp.arange(T)[None, :], (B, T))
        win = (
            jnp.full((B, G, G, A), -1, jnp.int32)
            .at[b_idx, gj_s, gi, best]
            .set(tnum, mode="drop")
        )
    return np.asarray(win)


def _prepare(bbox_prediction, landmarks_prediction, bbox_target, landmarks_target):
    """Host prep: target assignment + gather + bf16 packing."""
    bbox_prediction = np.asarray(bbox_prediction, dtype=np.float32)
    landmarks_prediction = np.asarray(landmarks_prediction, dtype=np.float32)
    bbox_target = np.asarray(bbox_target, dtype=np.float32)
    landmarks_target = np.asarray(landmarks_target, dtype=np.float32)

    win = _build_targets_host(bbox_target)
    cells = np.argwhere(win >= 0)                      # (n, 4): b, gj, gi, a
    twin = win[win >= 0]                               # aligned winners
    n_obj = len(cells)

    cb, cj, ci, ca = cells[:, 0], cells[:, 1], cells[:, 2], cells[:, 3]
    lmp_all = landmarks_prediction[cb, cj, ci, ca]     # (n, 68, 2)
    lmt_all = landmarks_target[cb, twin]               # (n, 68, 2)
    bbp_all = bbox_prediction[cb, cj, ci, ca, :4]      # (n, 4)
    bbt_all = np.log1p(bbox_target[cb, twin, :4]).astype(np.float32)
    conf_all = bbox_prediction[cb, cj, ci, ca, 4]      # (n,)
    w2_all = (np.float32(1.0) / (bbt_all[:, 2] * bbt_all[:, 3])).astype(np.float32)

    in_maps = []
    for c in range(NCORES):
        sel = (cb >= c * BPC) & (cb < (c + 1) * BPC)
        r = int(sel.sum())
        rows = np.zeros((ROWS, ROWC), BF16)
        rows[:r, 0:68] = lmp_all[sel][:, :, 0]
        rows[:r, 68:136] = lmp_all[sel][:, :, 1]
        rows[:r, 136:140] = bbp_all[sel]
        rows[:r, 140] = conf_all[sel]
        rows[:r, 141] = conf_all[sel]
        rows[:r, 142:210] = lmt_all[sel][:, :, 0]
        rows[:r, 210:278] = lmt_all[sel][:, :, 1]
        rows[:r, 278:282] = bbt_all[sel]
        rows[:r, 282] = 1.0
        w2c = np.zeros(ROWS, np.float32)
        w2c[:r] = w2_all[sel]
        rows.view(np.uint16)[:, 284:286] = w2c.view(np.uint16).reshape(ROWS, 2)
        confc = bbox_prediction[c * BPC:(c + 1) * BPC, :, :, :, 4].reshape(-1)
        conf_pad = np.zeros(ROWS * CONF_F, BF16)
        conf_pad[:CONF_N] = confc
        rows[:, 288:288 + CONF_F] = conf_pad.reshape(ROWS, CONF_F)
        in_maps.append({"rows": rows})
    return in_maps, n_obj


def _combine(results, n_obj):
    S = np.zeros(OUTC, np.float64)
    for r in results:
        S += r["out"].reshape(ROWS, OUTC).astype(np.float64).sum(axis=0)
    s_bsq = S[0:4].sum()
    s_cse = S[4]
    s_csq = S[5]
    s_q = S[6:14].sum()
    s_nme = S[14]
    s_slab = S[15]
    n = max(float(n_obj), 1.0)
    n_noobj = max(float(B * CELLS - n_obj), 1.0)
    nme = 2.0 * s_nme / (68.0 * n)
    loc = 5.0 * 0.5 * (s_bsq - s_q) / (n * 4.0)
    conf = 0.5 * (s_slab - s_csq) / n_noobj + s_cse / n
    return (np.float32(nme), np.float32(loc), np.float32(conf))


def _run_device(in_maps, trace=False):
    from concourse.bass_utils import run_bass_kernel_spmd
    nc = _get_nc()
    return run_bass_kernel_spmd(nc, in_maps, list(range(NCORES)), trace=trace)


def kernel(bbox_prediction, landmarks_prediction, bbox_target, landmarks_target):
    in_maps, n_obj = _prepare(
        bbox_prediction, landmarks_prediction, bbox_target, landmarks_target)
    res = _run_device(in_maps)
    return _combine(res.results, n_obj)


# revision 36
# speedup vs baseline: 1.5324x; 1.0221x over previous
"""JointLoss (YOLO-style bbox + landmarks + confidence) on 8 Trainium2 cores.

v2: same host/device split as the baseline (host does target assignment +
gather, device computes the three losses), restructured for latency:

- Inputs ship as bf16 (tolerance 2e-2; accumulations stay f32), merged into
  two DMAs: `rows` (gathered obj-cell rows, issued by SP) and `conf` (dense
  conf channel, issued by ACT in parallel -- HWDGE paths don't serialize).
- Landmarks are packed x-block|y-block so the pair-sum is a packed 2x-mode
  tensor_tensor instead of a strided add.
- Smooth-L1 uses the identity  sl1(d) = 0.5*d^2 - 0.5*relu(|d|-1)^2, with
  q = (|d| abs_max 1) - 1 computed in ONE fused tensor_scalar, so the whole
  bbox term is two small DVE ops squeezed into the shadow of the ACT sqrt.
- The critical DVE chain is only sub -> square -> pairadd (3 levels).
- Output DMA is a Pool-engine PREPARED kv_writeback: descriptors are
  generated during the input-DMA dead window; when the partials are ready a
  cheap trigger_dma fires it, skipping the ~1.3us HWDGE+DGE launch path.
- Per-partition partials [128,12] go back; host reduces in f64.
"""

import numpy as np
import ml_dtypes

BF16 = ml_dtypes.bfloat16

B, T, G, A = 32, 32, 36, 5
NCORES = 8
BPC = B // NCORES            # batches per core
CELLS = G * G * A            # 6480 per batch
ROWS = 128                   # padded obj rows per core (max B/NCORES*T)
CONF_N = BPC * CELLS         # 25920 dense conf elems per core
CONF_F = 256                 # gather elem: 256 bf16 = 512B (must be %256B)
# rows cols (bf16): a 0:142 | b 142:284 | w2(f32) 284:286 | pad
ROWC = 288
OUTC = 16                    # out cols: bd^2(4) cm1^2 cconf^2 r1^2(4) r2^2(4) nme slab

IMAGE_SIZE = 288.0
ANCHORS = np.array([[0.24, 0.24], [0.12, 0.12], [0.08, 0.08],
                    [0.28, 0.28], [0.15, 0.15]], dtype=np.float32)

_STATE = {}

# Output path: True = Pool-prepared kv_writeback + trigger_dma,
# False = plain SP HWDGE dma_start (fallback).
OUT_VIA_TRIGGER = True
# Dense conf channel: True = Pool-prepared dma_gather (early delivery),
# False = ACT-issued plain DMA.
CONF_VIA_GATHER = True
# Strip PE's preamble RegisterMoves (PE runs no engine ops).
STRIP_PE_REGMOVES = True
# Attach no semaphore wait to the output DMA (NRT drains DMA queues at
# kernel end); False adds an SP-side wait on the completion sem.
NO_FINAL_WAIT = True


def _build_program():
    import concourse.bass as bass
    from concourse import mybir
    from concourse import library_config
    from contextlib import ExitStack

    nc = bass.Bass()
    f32 = mybir.dt.float32
    bf16 = mybir.dt.bfloat16
    i32 = mybir.dt.int32
    op = mybir.AluOpType
    fn = mybir.ActivationFunctionType

    # Drop the framework's const-AP memsets — nothing reads them (we pass
    # explicit zero-bias APs); they serialize on Pool ahead of the initial
    # all-engine barrier.
    _unused = {"const-float32-0.0", "const-float32-1.0",
               "const-bfloat16-1.0", "const-uint8-127"}
    for _bb in nc.m.functions[0].blocks:
        _keep = []
        for _ins in _bb.instructions:
            _drop = (
                str(_ins.opcode) in ("Memset", "InstructionName.Memset")
                and _ins.outs
                and any(u in str(_ins.outs[0]) for u in _unused)
            ) or (
                # PE executes no engine ops; its register inits only delay
                # the initial barrier (PE is the slowest follower).
                STRIP_PE_REGMOVES
                and str(_ins.opcode).endswith("RegisterMove")
                and str(_ins.engine).endswith("PE")
            )
            if _drop:
                del nc.inst_map[_ins.name]
            else:
                _keep.append(_ins)
        if len(_keep) != len(_bb.instructions):
            _bb.instructions[:] = _keep

    rows_p = nc.declare_dram_parameter("rows", [ROWS, ROWC], bf16, isOutput=False)
    conf_p = nc.declare_dram_parameter("conf", [ROWS, CONF_F], bf16, isOutput=False)
    out_p = nc.declare_dram_parameter("out", [1, ROWS, 1, OUTC], f32, isOutput=True)

    st = ExitStack()
    Tt = lambda n, s, d: st.enter_context(nc.sbuf_tensor(n, s, d))
    rows_t = Tt("rows_t", [ROWS, ROWC], bf16)
    conf_t = Tt("conf_t", [ROWS, 1, CONF_F], bf16)
    sub_t = Tt("sub_t", [ROWS, 150], bf16)   # ldx ldy | bd 136:140 | cm1 | cconf | r1 142:146 | r2 146:150
    sq_t = Tt("sq_t", [ROWS, 136], bf16)
    ps_t = Tt("ps_t", [ROWS, 68], bf16)
    dj_t = Tt("dj_t", [ROWS, 68], bf16)      # sqrt elementwise out (junk)
    cj_t = Tt("cj_t", [ROWS, CONF_F], bf16)  # conf^2 elementwise out (junk)
    out_t = Tt("out_t", [ROWS, 1, 1, OUTC], f32)
    ctx_t = Tt("ctx_t", [ROWS, 1], i32)
    idx_t = Tt("idx_t", [ROWS, 8], mybir.dt.int16)

    def o2(a, b):            # 2-D [128, b-a] view of out_t cols
        return out_t[:, 0:1, 0:1, a:b].squeeze(1).squeeze(1)

    w2_ap = rows_t[:, 284:286].bitcast(f32)  # [128,1] f32 NME scale
    # Explicit zero biases (host-packed pad columns), each sourced from the
    # same tensor the activation already depends on.
    zb_rows = rows_t[:, 286:288].bitcast(f32)
    zb_conf = conf_t[:, 0:1, 204:206].squeeze(1).bitcast(f32)

    with nc.Block() as block, \
            nc.semaphore("dsa") as dsa, \
            nc.semaphore("dsb") as dsb, \
            nc.semaphore("vsem") as vsem, \
            nc.semaphore("csem") as csem, \
            nc.semaphore("psem") as psem, \
            nc.semaphore("osem") as osem:

        @block.sync
        def _(sync):
            sync.dma_start(out=rows_t[:], in_=rows_p[:]).then_inc(dsa, 16)
            if not OUT_VIA_TRIGGER:
                sync.wait_ge(csem, 2)
                sync.dma_start(out=out_p[:], in_=out_t[:]).then_inc(osem, 16)
                if not NO_FINAL_WAIT:
                    sync.wait_ge(osem, 16)
            elif not NO_FINAL_WAIT:
                sync.wait_ge(osem, 16)

        @block.scalar
        def _(scalar):
            if not (OUT_VIA_TRIGGER and CONF_VIA_GATHER):
                scalar.dma_start(out=conf_t[:], in_=conf_p[:]).then_inc(dsb, 16)
            scalar.activation(out=cj_t[:, 0:204],
                              in_=conf_t[:, 0:1, 0:204].squeeze(1),
                              func=fn.Square, bias=zb_conf,
                              accum_out=o2(15, 16),
                              )._wait_ge(dsb, 16)
            scalar.activation(out=dj_t[:], in_=ps_t[:], func=fn.Sqrt,
                              scale=w2_ap, bias=zb_rows, accum_out=o2(14, 15),
                              )._wait_ge(vsem, 1)
            scalar.drain().then_inc(csem, 1)

        @block.vector
        def _(vector):
            vector.tensor_tensor(out=sub_t[:, 0:142], in0=rows_t[:, 0:142],
                                 in1=rows_t[:, 142:284], op=op.subtract,
                                 )._wait_ge(dsa, 16)
            vector.drain()
            vector.tensor_tensor(out=sq_t[:], in0=sub_t[:, 0:136],
                                 in1=sub_t[:, 0:136], op=op.mult)
            vector.drain()
            vector.tensor_tensor(out=ps_t[:], in0=sq_t[:, 0:68],
                                 in1=sq_t[:, 68:136], op=op.add)
            vector.drain().then_inc(vsem, 1)
            # sl1 = 0.5*d^2 - 0.5*relu(|d|-1)^2; relu(|d|-1)^2 = r1^2 + r2^2
            # with r1 = relu(d-1) = (d-1) max 0, r2 = min(d+1, 0).
            vector.tensor_scalar(out=sub_t[:, 142:146], in0=sub_t[:, 136:140],
                                 scalar1=1.0, scalar2=0.0,
                                 op0=op.subtract, op1=op.max)
            vector.tensor_scalar(out=sub_t[:, 146:150], in0=sub_t[:, 136:140],
                                 scalar1=1.0, scalar2=0.0,
                                 op0=op.add, op1=op.min)
            vector.drain()
            vector.tensor_tensor(out=o2(0, 14), in0=sub_t[:, 136:150],
                                 in1=sub_t[:, 136:150], op=op.mult)
            vector.drain().then_inc(csem, 1)

        if OUT_VIA_TRIGGER:
            @block.gpsimd
            def _(gpsimd):
                # bass_rust hardcodes the older opcode numbering (235 = this
                # toolchain's HINT); rewrite to this ISA's TRIGGER_DMA.
                _trig_op = int(
                    nc.isa.Opcode.NEURON_ISA_TPB_OPCODE_TRIGGER_DMA.value)
                npre = 0
                if CONF_VIA_GATHER:
                    # The gather ucode reads idx k from partition 16 + k%16,
                    # col k//16 (measured); iota needs the standard library.
                    gpsimd.load_library(library_config.standard)
                    gpsimd.iota(idx_t[:, :], pattern=[[16, 8]], base=-16,
                                channel_multiplier=1)
                    gpsimd.load_library(library_config.attnmlp)
                    gpsimd.dma_gather(
                        out_ap=conf_t[:], in_ap=conf_p[:],
                        idxs_ap=idx_t[0:16, :],
                        num_idxs=ROWS, num_idxs_reg=ROWS, elem_size=CONF_F,
                        prepare_only=True, sem=dsb,
                    ).then_inc(psem, 1)
                    gpsimd.wait_ge(psem, 1)
                    trig1 = gpsimd.trigger_dma(count=1)
                    trig1.ins.isa_opcode = _trig_op
                    npre = 1
                else:
                    gpsimd.load_library(library_config.attn)
                gpsimd.memset(ctx_t[:], 0)
                gpsimd.kv_writeback(
                    out_ap=out_p[:], in_ap=out_t[:], ctx_idxs_ap=ctx_t[:],
                    prepare_only=True, sem=osem,
                ).then_inc(psem, 1)
                gpsimd.wait_ge(psem, npre + 1)
                gpsimd.wait_ge(csem, 2)
                trig2 = gpsimd.trigger_dma(count=1)
                trig2.ins.isa_opcode = _trig_op

    st.close()
    # Raw Bass skips Bacc's ISA-subclass lowering; run it so the trigger /
    # library-reload pseudo instructions get real ISA bytes for walrus.
    mybir.codegen_inst_isa_subclasses(nc)
    return nc


def _get_nc():
    if "nc" not in _STATE:
        _STATE["nc"] = _build_program()
    return _STATE["nc"]


def _build_targets_host(bbox_target):
    """Replicate reference build_targets' cell assignment exactly (jax-CPU),
    returning the winning target index per grid cell (-1 = no object)."""
    import jax
    import jax.numpy as jnp

    cpu = jax.devices("cpu")[0]
    with jax.default_device(cpu):
        bt = jnp.asarray(np.asarray(bbox_target), dtype=jnp.float32)
        gt = bt[..., :4]
        valid = jnp.sum(bt, axis=-1) != 0
        gi = (gt[..., 0] * G).astype(jnp.int32)
        gj = (gt[..., 1] * G).astype(jnp.int32)
        acx = (0.5 + gi.astype(gt.dtype)) / G
        acy = (0.5 + gj.astype(gt.dtype)) / G
        aw = jnp.asarray(ANCHORS)[:, 0]
        ah = jnp.asarray(ANCHORS)[:, 1]

        def corners(cx, cy, w, h):
            x1 = (cx - w / 2) * IMAGE_SIZE
            x2 = (cx + w / 2) * IMAGE_SIZE
            y1 = (cy - h / 2) * IMAGE_SIZE
            y2 = (cy + h / 2) * IMAGE_SIZE
            return x1, x2, y1, y2

        gx1, gx2, gy1, gy2 = corners(gt[..., 0], gt[..., 1], gt[..., 2], gt[..., 3])
        ax1, ax2, ay1, ay2 = corners(acx[..., None], acy[..., None], aw, ah)
        ix1 = jnp.maximum(gx1[..., None], ax1)
        iy1 = jnp.maximum(gy1[..., None], ay1)
        ix2 = jnp.minimum(gx2[..., None], ax2)
        iy2 = jnp.minimum(gy2[..., None], ay2)
        inter = (ix2 - ix1 + 1) * (iy2 - iy1 + 1)
        area_g = ((gx2 - gx1 + 1) * (gy2 - gy1 + 1))[..., None]
        area_a = (ax2 - ax1 + 1) * (ay2 - ay1 + 1)
        iou = inter / (area_g + area_a - inter + 1e-16)
        best = jnp.argmax(iou, axis=-1)
        b_idx = jnp.broadcast_to(jnp.arange(B)[:, None], (B, T))
        gj_s = jnp.where(valid, gj, G)
        tnum = jnp.broadcast_to(jnp.arange(T)[None, :], (B, T))
        win = (
            jnp.full((B, G, G, A), -1, jnp.int32)
            .at[b_idx, gj_s, gi, best]
            .set(tnum, mode="drop")
        )
    return np.asarray(win)


def _prepare(bbox_prediction, landmarks_prediction, bbox_target, landmarks_target):
    """Host prep: target assignment + gather + bf16 packing."""
    bbox_prediction = np.asarray(bbox_prediction, dtype=np.float32)
    landmarks_prediction = np.asarray(landmarks_prediction, dtype=np.float32)
    bbox_target = np.asarray(bbox_target, dtype=np.float32)
    landmarks_target = np.asarray(landmarks_target, dtype=np.float32)

    win = _build_targets_host(bbox_target)
    cells = np.argwhere(win >= 0)                      # (n, 4): b, gj, gi, a
    twin = win[win >= 0]                               # aligned winners
    n_obj = len(cells)

    cb, cj, ci, ca = cells[:, 0], cells[:, 1], cells[:, 2], cells[:, 3]
    lmp_all = landmarks_prediction[cb, cj, ci, ca]     # (n, 68, 2)
    lmt_all = landmarks_target[cb, twin]               # (n, 68, 2)
    bbp_all = bbox_prediction[cb, cj, ci, ca, :4]      # (n, 4)
    bbt_all = np.log1p(bbox_target[cb, twin, :4]).astype(np.float32)
    conf_all = bbox_prediction[cb, cj, ci, ca, 4]      # (n,)
    w2_all = (np.float32(1.0) / (bbt_all[:, 2] * bbt_all[:, 3])).astype(np.float32)

    in_maps = []
    for c in range(NCORES):
        sel = (cb >= c * BPC) & (cb < (c + 1) * BPC)
        r = int(sel.sum())
        rows = np.zeros((ROWS, ROWC), BF16)
        rows[:r, 0:68] = lmp_all[sel][:, :, 0]
        rows[:r, 68:136] = lmp_all[sel][:, :, 1]
        rows[:r, 136:140] = bbp_all[sel]
        rows[:r, 140] = conf_all[sel]
        rows[:r, 141] = conf_all[sel]
        rows[:r, 142:210] = lmt_all[sel][:, :, 0]
        rows[:r, 210:278] = lmt_all[sel][:, :, 1]
        rows[:r, 278:282] = bbt_all[sel]
        rows[:r, 282] = 1.0
        w2c = np.zeros(ROWS, np.float32)
        w2c[:r] = w2_all[sel]
        rows.view(np.uint16)[:, 284:286] = w2c.view(np.uint16).reshape(ROWS, 2)
        confc = bbox_prediction[c * BPC:(c + 1) * BPC, :, :, :, 4].reshape(-1)
        conf2d = np.zeros((ROWS, CONF_F), BF16)
        cpad = np.zeros(ROWS * 204, np.float32)
        cpad[:CONF_N] = confc
        conf2d[:, 0:204] = cpad.reshape(ROWS, 204)
        in_maps.append({"rows": rows, "conf": conf2d})
    return in_maps, n_obj


def _combine(results, n_obj):
    S = np.zeros(OUTC, np.float64)
    for r in results:
        S += r["out"].reshape(ROWS, OUTC).astype(np.float64).sum(axis=0)
    s_bsq = S[0:4].sum()
    s_cse = S[4]
    s_csq = S[5]
    s_q = S[6:14].sum()
    s_nme = S[14]
    s_slab = S[15]
    n = max(float(n_obj), 1.0)
    n_noobj = max(float(B * CELLS - n_obj), 1.0)
    nme = 2.0 * s_nme / (68.0 * n)
    loc = 5.0 * 0.5 * (s_bsq - s_q) / (n * 4.0)
    conf = 0.5 * (s_slab - s_csq) / n_noobj + s_cse / n
    return (np.float32(nme), np.float32(loc), np.float32(conf))


def _run_device(in_maps, trace=False):
    from concourse.bass_utils import run_bass_kernel_spmd
    nc = _get_nc()
    return run_bass_kernel_spmd(nc, in_maps, list(range(NCORES)), trace=trace)


def kernel(bbox_prediction, landmarks_prediction, bbox_target, landmarks_target):
    in_maps, n_obj = _prepare(
        bbox_prediction, landmarks_prediction, bbox_target, landmarks_target)
    res = _run_device(in_maps)
    return _combine(res.results, n_obj)


# revision 38
# speedup vs baseline: 1.6358x; 1.0674x over previous
"""JointLoss (YOLO-style bbox + landmarks + confidence) on 8 Trainium2 cores.

v2: same host/device split as the baseline (host does target assignment +
gather, device computes the three losses), restructured for latency:

- Inputs ship as bf16 (tolerance 2e-2; accumulations stay f32), merged into
  two DMAs: `rows` (gathered obj-cell rows, issued by SP) and `conf` (dense
  conf channel, issued by ACT in parallel -- HWDGE paths don't serialize).
- Landmarks are packed x-block|y-block so the pair-sum is a packed 2x-mode
  tensor_tensor instead of a strided add.
- Smooth-L1 uses the identity  sl1(d) = 0.5*d^2 - 0.5*relu(|d|-1)^2, with
  q = (|d| abs_max 1) - 1 computed in ONE fused tensor_scalar, so the whole
  bbox term is two small DVE ops squeezed into the shadow of the ACT sqrt.
- The critical DVE chain is only sub -> square -> pairadd (3 levels).
- Output DMA is a Pool-engine PREPARED kv_writeback: descriptors are
  generated during the input-DMA dead window; when the partials are ready a
  cheap trigger_dma fires it, skipping the ~1.3us HWDGE+DGE launch path.
- Per-partition partials [128,12] go back; host reduces in f64.
"""

import numpy as np
import ml_dtypes

BF16 = ml_dtypes.bfloat16

B, T, G, A = 32, 32, 36, 5
NCORES = 8
BPC = B // NCORES            # batches per core
CELLS = G * G * A            # 6480 per batch
ROWS = 128                   # padded obj rows per core (max B/NCORES*T)
CONF_N = BPC * CELLS         # 25920 dense conf elems per core
CONF_F = 256                 # gather elem: 256 bf16 = 512B (must be %256B)
# rows cols (bf16): a 0:142 | b 142:284 | w2(f32) 284:286 | pad
ROWC = 288
OUTC = 16                    # out cols: bd^2(4) cm1^2 cconf^2 r1^2(4) r2^2(4) nme slab

IMAGE_SIZE = 288.0
ANCHORS = np.array([[0.24, 0.24], [0.12, 0.12], [0.08, 0.08],
                    [0.28, 0.28], [0.15, 0.15]], dtype=np.float32)

_STATE = {}

# Output path: True = Pool-prepared kv_writeback + trigger_dma,
# False = plain SP HWDGE dma_start (fallback).
OUT_VIA_TRIGGER = True
# Dense conf channel: True = Pool-prepared dma_gather (early delivery),
# False = ACT-issued plain DMA.
CONF_VIA_GATHER = True
# Strip preamble RegisterMoves: "pe" = only PE's (PE runs no engine ops),
# "all" = every engine's (registers are runtime-reset; bcreg/zero unused).
STRIP_REGMOVES = "all"
# Attach no semaphore wait to the output DMA (NRT drains DMA queues at
# kernel end); False adds an SP-side wait on the completion sem.
NO_FINAL_WAIT = True


def _build_program():
    import concourse.bass as bass
    from concourse import mybir
    from concourse import library_config
    from contextlib import ExitStack

    nc = bass.Bass()
    f32 = mybir.dt.float32
    bf16 = mybir.dt.bfloat16
    i32 = mybir.dt.int32
    op = mybir.AluOpType
    fn = mybir.ActivationFunctionType

    # Drop the framework's const-AP memsets — nothing reads them (we pass
    # explicit zero-bias APs); they serialize on Pool ahead of the initial
    # all-engine barrier.
    _unused = {"const-float32-0.0", "const-float32-1.0",
               "const-bfloat16-1.0", "const-uint8-127"}
    for _bb in nc.m.functions[0].blocks:
        _keep = []
        for _ins in _bb.instructions:
            _drop = (
                str(_ins.opcode) in ("Memset", "InstructionName.Memset")
                and _ins.outs
                and any(u in str(_ins.outs[0]) for u in _unused)
            ) or (
                str(_ins.opcode).endswith("RegisterMove")
                and (STRIP_REGMOVES == "all"
                     or (STRIP_REGMOVES == "pe"
                         and str(_ins.engine).endswith("PE")))
            )
            if _drop:
                del nc.inst_map[_ins.name]
            else:
                _keep.append(_ins)
        if len(_keep) != len(_bb.instructions):
            _bb.instructions[:] = _keep

    rows_p = nc.declare_dram_parameter("rows", [ROWS, ROWC], bf16, isOutput=False)
    conf_p = nc.declare_dram_parameter("conf", [ROWS, CONF_F], bf16, isOutput=False)
    out_p = nc.declare_dram_parameter("out", [1, ROWS, 1, OUTC], f32, isOutput=True)

    st = ExitStack()
    Tt = lambda n, s, d: st.enter_context(nc.sbuf_tensor(n, s, d))
    rows_t = Tt("rows_t", [ROWS, ROWC], bf16)
    conf_t = Tt("conf_t", [ROWS, 1, CONF_F], bf16)
    sub_t = Tt("sub_t", [ROWS, 150], bf16)   # ldx ldy | bd 136:140 | cm1 | cconf | r1 142:146 | r2 146:150
    sq_t = Tt("sq_t", [ROWS, 136], bf16)
    ps_t = Tt("ps_t", [ROWS, 68], bf16)
    dj_t = Tt("dj_t", [ROWS, 68], bf16)      # sqrt elementwise out (junk)
    cj_t = Tt("cj_t", [ROWS, CONF_F], bf16)  # conf^2 elementwise out (junk)
    out_t = Tt("out_t", [ROWS, 1, 1, OUTC], f32)
    ctx_t = Tt("ctx_t", [ROWS, 1], i32)
    idx_t = Tt("idx_t", [ROWS, 8], mybir.dt.int16)

    def o2(a, b):            # 2-D [128, b-a] view of out_t cols
        return out_t[:, 0:1, 0:1, a:b].squeeze(1).squeeze(1)

    w2_ap = rows_t[:, 284:286].bitcast(f32)  # [128,1] f32 NME scale
    # Explicit zero biases (host-packed pad columns), each sourced from the
    # same tensor the activation already depends on.
    zb_rows = rows_t[:, 286:288].bitcast(f32)
    zb_conf = conf_t[:, 0:1, 204:206].squeeze(1).bitcast(f32)

    with nc.Block() as block, \
            nc.semaphore("dsa") as dsa, \
            nc.semaphore("dsb") as dsb, \
            nc.semaphore("vsem") as vsem, \
            nc.semaphore("csem") as csem, \
            nc.semaphore("psem") as psem, \
            nc.semaphore("osem") as osem:

        @block.sync
        def _(sync):
            sync.dma_start(out=rows_t[:], in_=rows_p[:]).then_inc(dsa, 16)
            if not OUT_VIA_TRIGGER:
                sync.wait_ge(csem, 2)
                sync.dma_start(out=out_p[:], in_=out_t[:]).then_inc(osem, 16)
                if not NO_FINAL_WAIT:
                    sync.wait_ge(osem, 16)
            elif not NO_FINAL_WAIT:
                sync.wait_ge(osem, 16)

        @block.scalar
        def _(scalar):
            if not (OUT_VIA_TRIGGER and CONF_VIA_GATHER):
                scalar.dma_start(out=conf_t[:], in_=conf_p[:]).then_inc(dsb, 16)
            scalar.activation(out=cj_t[:, 0:204],
                              in_=conf_t[:, 0:1, 0:204].squeeze(1),
                              func=fn.Square, bias=zb_conf,
                              accum_out=o2(15, 16),
                              )._wait_ge(dsb, 16)
            scalar.activation(out=dj_t[:], in_=ps_t[:], func=fn.Sqrt,
                              scale=w2_ap, bias=zb_rows, accum_out=o2(14, 15),
                              )._wait_ge(vsem, 1)
            scalar.drain().then_inc(csem, 1)

        @block.vector
        def _(vector):
            vector.tensor_tensor(out=sub_t[:, 0:142], in0=rows_t[:, 0:142],
                                 in1=rows_t[:, 142:284], op=op.subtract,
                                 )._wait_ge(dsa, 16)
            vector.drain()
            vector.tensor_tensor(out=sq_t[:], in0=sub_t[:, 0:136],
                                 in1=sub_t[:, 0:136], op=op.mult)
            vector.drain()
            vector.tensor_tensor(out=ps_t[:], in0=sq_t[:, 0:68],
                                 in1=sq_t[:, 68:136], op=op.add)
            vector.drain().then_inc(vsem, 1)
            # sl1 = 0.5*d^2 - 0.5*relu(|d|-1)^2; relu(|d|-1)^2 = r1^2 + r2^2
            # with r1 = relu(d-1) = (d-1) max 0, r2 = min(d+1, 0).
            vector.tensor_scalar(out=sub_t[:, 142:146], in0=sub_t[:, 136:140],
                                 scalar1=1.0, scalar2=0.0,
                                 op0=op.subtract, op1=op.max)
            vector.tensor_scalar(out=sub_t[:, 146:150], in0=sub_t[:, 136:140],
                                 scalar1=1.0, scalar2=0.0,
                                 op0=op.add, op1=op.min)
            vector.drain()
            vector.tensor_tensor(out=o2(0, 14), in0=sub_t[:, 136:150],
                                 in1=sub_t[:, 136:150], op=op.mult)
            vector.drain().then_inc(csem, 1)

        if OUT_VIA_TRIGGER:
            @block.gpsimd
            def _(gpsimd):
                # bass_rust hardcodes the older opcode numbering (235 = this
                # toolchain's HINT); rewrite to this ISA's TRIGGER_DMA.
                _trig_op = int(
                    nc.isa.Opcode.NEURON_ISA_TPB_OPCODE_TRIGGER_DMA.value)
                npre = 0
                if CONF_VIA_GATHER:
                    # The gather ucode reads idx k from partition 16 + k%16,
                    # col k//16 (measured); iota needs the standard library.
                    gpsimd.load_library(library_config.standard)
                    gpsimd.iota(idx_t[:, :], pattern=[[16, 8]], base=-16,
                                channel_multiplier=1)
                    gpsimd.load_library(library_config.attnmlp)
                    gpsimd.dma_gather(
                        out_ap=conf_t[:], in_ap=conf_p[:],
                        idxs_ap=idx_t[0:16, :],
                        num_idxs=ROWS, num_idxs_reg=ROWS, elem_size=CONF_F,
                        prepare_only=True, sem=dsb,
                    ).then_inc(psem, 1)
                    gpsimd.wait_ge(psem, 1)
                    trig1 = gpsimd.trigger_dma(count=1)
                    trig1.ins.isa_opcode = _trig_op
                    npre = 1
                else:
                    gpsimd.load_library(library_config.attn)
                gpsimd.memset(ctx_t[:], 0)
                gpsimd.kv_writeback(
                    out_ap=out_p[:], in_ap=out_t[:], ctx_idxs_ap=ctx_t[:],
                    prepare_only=True, sem=osem,
                ).then_inc(psem, 1)
                gpsimd.wait_ge(psem, npre + 1)
                gpsimd.wait_ge(csem, 2)
                trig2 = gpsimd.trigger_dma(count=1)
                trig2.ins.isa_opcode = _trig_op

    st.close()
    # Raw Bass skips Bacc's ISA-subclass lowering; run it so the trigger /
    # library-reload pseudo instructions get real ISA bytes for walrus.
    mybir.codegen_inst_isa_subclasses(nc)
    return nc


def _get_nc():
    if "nc" not in _STATE:
        _STATE["nc"] = _build_program()
    return _STATE["nc"]


def _build_targets_host(bbox_target):
    """Replicate reference build_targets' cell assignment exactly (jax-CPU),
    returning the winning target index per grid cell (-1 = no object)."""
    import jax
    import jax.numpy as jnp

    cpu = jax.devices("cpu")[0]
    with jax.default_device(cpu):
        bt = jnp.asarray(np.asarray(bbox_target), dtype=jnp.float32)
        gt = bt[..., :4]
        valid = jnp.sum(bt, axis=-1) != 0
        gi = (gt[..., 0] * G).astype(jnp.int32)
        gj = (gt[..., 1] * G).astype(jnp.int32)
        acx = (0.5 + gi.astype(gt.dtype)) / G
        acy = (0.5 + gj.astype(gt.dtype)) / G
        aw = jnp.asarray(ANCHORS)[:, 0]
        ah = jnp.asarray(ANCHORS)[:, 1]

        def corners(cx, cy, w, h):
            x1 = (cx - w / 2) * IMAGE_SIZE
            x2 = (cx + w / 2) * IMAGE_SIZE
            y1 = (cy - h / 2) * IMAGE_SIZE
            y2 = (cy + h / 2) * IMAGE_SIZE
            return x1, x2, y1, y2

        gx1, gx2, gy1, gy2 = corners(gt[..., 0], gt[..., 1], gt[..., 2], gt[..., 3])
        ax1, ax2, ay1, ay2 = corners(acx[..., None], acy[..., None], aw, ah)
        ix1 = jnp.maximum(gx1[..., None], ax1)
        iy1 = jnp.maximum(gy1[..., None], ay1)
        ix2 = jnp.minimum(gx2[..., None], ax2)
        iy2 = jnp.minimum(gy2[..., None], ay2)
        inter = (ix2 - ix1 + 1) * (iy2 - iy1 + 1)
        area_g = ((gx2 - gx1 + 1) * (gy2 - gy1 + 1))[..., None]
        area_a = (ax2 - ax1 + 1) * (ay2 - ay1 + 1)
        iou = inter / (area_g + area_a - inter + 1e-16)
        best = jnp.argmax(iou, axis=-1)
        b_idx = jnp.broadcast_to(jnp.arange(B)[:, None], (B, T))
        gj_s = jnp.where(valid, gj, G)
        tnum = jnp.broadcast_to(jnp.arange(T)[None, :], (B, T))
        win = (
            jnp.full((B, G, G, A), -1, jnp.int32)
            .at[b_idx, gj_s, gi, best]
            .set(tnum, mode="drop")
        )
    return np.asarray(win)


def _prepare(bbox_prediction, landmarks_prediction, bbox_target, landmarks_target):
    """Host prep: target assignment + gather + bf16 packing."""
    bbox_prediction = np.asarray(bbox_prediction, dtype=np.float32)
    landmarks_prediction = np.asarray(landmarks_prediction, dtype=np.float32)
    bbox_target = np.asarray(bbox_target, dtype=np.float32)
    landmarks_target = np.asarray(landmarks_target, dtype=np.float32)

    win = _build_targets_host(bbox_target)
    cells = np.argwhere(win >= 0)                      # (n, 4): b, gj, gi, a
    twin = win[win >= 0]                               # aligned winners
    n_obj = len(cells)

    cb, cj, ci, ca = cells[:, 0], cells[:, 1], cells[:, 2], cells[:, 3]
    lmp_all = landmarks_prediction[cb, cj, ci, ca]     # (n, 68, 2)
    lmt_all = landmarks_target[cb, twin]               # (n, 68, 2)
    bbp_all = bbox_prediction[cb, cj, ci, ca, :4]      # (n, 4)
    bbt_all = np.log1p(bbox_target[cb, twin, :4]).astype(np.float32)
    conf_all = bbox_prediction[cb, cj, ci, ca, 4]      # (n,)
    w2_all = (np.float32(1.0) / (bbt_all[:, 2] * bbt_all[:, 3])).astype(np.float32)

    in_maps = []
    for c in range(NCORES):
        sel = (cb >= c * BPC) & (cb < (c + 1) * BPC)
        r = int(sel.sum())
        rows = np.zeros((ROWS, ROWC), BF16)
        rows[:r, 0:68] = lmp_all[sel][:, :, 0]
        rows[:r, 68:136] = lmp_all[sel][:, :, 1]
        rows[:r, 136:140] = bbp_all[sel]
        rows[:r, 140] = conf_all[sel]
        rows[:r, 141] = conf_all[sel]
        rows[:r, 142:210] = lmt_all[sel][:, :, 0]
        rows[:r, 210:278] = lmt_all[sel][:, :, 1]
        rows[:r, 278:282] = bbt_all[sel]
        rows[:r, 282] = 1.0
        w2c = np.zeros(ROWS, np.float32)
        w2c[:r] = w2_all[sel]
        rows.view(np.uint16)[:, 284:286] = w2c.view(np.uint16).reshape(ROWS, 2)
        confc = bbox_prediction[c * BPC:(c + 1) * BPC, :, :, :, 4].reshape(-1)
        conf2d = np.zeros((ROWS, CONF_F), BF16)
        cpad = np.zeros(ROWS * 204, np.float32)
        cpad[:CONF_N] = confc
        conf2d[:, 0:204] = cpad.reshape(ROWS, 204)
        in_maps.append({"rows": rows, "conf": conf2d})
    return in_maps, n_obj


def _combine(results, n_obj):
    S = np.zeros(OUTC, np.float64)
    for r in results:
        S += r["out"].reshape(ROWS, OUTC).astype(np.float64).sum(axis=0)
    s_bsq = S[0:4].sum()
    s_cse = S[4]
    s_csq = S[5]
    s_q = S[6:14].sum()
    s_nme = S[14]
    s_slab = S[15]
    n = max(float(n_obj), 1.0)
    n_noobj = max(float(B * CELLS - n_obj), 1.0)
    nme = 2.0 * s_nme / (68.0 * n)
    loc = 5.0 * 0.5 * (s_bsq - s_q) / (n * 4.0)
    conf = 0.5 * (s_slab - s_csq) / n_noobj + s_cse / n
    return (np.float32(nme), np.float32(loc), np.float32(conf))


def _run_device(in_maps, trace=False):
    from concourse.bass_utils import run_bass_kernel_spmd
    nc = _get_nc()
    return run_bass_kernel_spmd(nc, in_maps, list(range(NCORES)), trace=trace)


def kernel(bbox_prediction, landmarks_prediction, bbox_target, landmarks_target):
    in_maps, n_obj = _prepare(
        bbox_prediction, landmarks_prediction, bbox_target, landmarks_target)
    res = _run_device(in_maps)
    return _combine(res.results, n_obj)
